# revision 43
# baseline (speedup 1.0000x reference)
"""IntLoRA-SHIFT fused kernel for Trainium2 (8 NeuronCores, tensor-parallel on out_features).

Math (per reference):
    w_int  = ori_weight_round - zero_point                    [O, I]
    lora   = (aux_R + loraB @ loraA) / where(w_int==0, 1, w_int)
    wu     = delta + lora
    weight = sign(wu) * 2^round(log2|wu|) * w_int
    out    = x @ weight.T + bias

Kernel strategy per core (O sharded 8 x 512):
  Everything lives in [i(partition), o(free)] layout so weight prep emits the
  k-tile-transposed weight wT directly (no on-device transposes).  The host
  pre-transposes ori (uint8) and aux (f32) per core; per-o params
  (zero_point/delta/bias) are partition-broadcast once into [128, osh] tiles.

  Phase A (per i-chunk k, [128, 512] tiles, 7 DVE passes, all other engines
  free):
    PE:  ba = (loraB@loraA)^T slice  (K=4 bf16 matmul into PSUM)
    DVE: den = (ori + 1e-4) - zp_b      (epsilon keeps den != 0 at w_int==0;
         q*den still rounds to the exact bf16 weight for w_int != 0)
    DVE: rcp ~= 1/den (reciprocal_approx_fast, ~18 bits -- only the rounded
         log2 of the quotient is needed)
    DVE: num = (aux + ba) * rcp ; num = num*sqrt2 + delta*sqrt2   (fused stt;
         exponent-floor of wu*sqrt2 == round(log2|wu|))
    DVE: q = bits(num) >>23 <<23 (fused shift pair: sign * 2^round(log2|wu|))
    DVE: wT[:,k,:] = q * den  (exact in bf16: |w_int|<=255 has 8 significand bits)

  Phase B: host-transposed bf16 x streamed as [128, nk, 512] groups (contig
  32KB/partition DMAs, triple-buffered), 32 bf16 matmuls per 128-token tile
  accumulate in PSUM rotating over 7 banks (bank-level ILP on the PE is worth
  ~2x vs 4 banks), drained by DVE bias-adds and stored with HWDGE DMAs.
  All matmuls are bf16: no fp32/bf16 PE mode switches anywhere.
"""
import os
import sys

for _p in ("/root/.axon_site", "/root/.axon_site/_ro/trn_rl_repo", "/root/.axon_site/_ro/pypackages", "/opt/trn_rl_repo"):
    if os.path.isdir(_p) and _p not in sys.path:
        sys.path.append(_p)

import numpy as np

import concourse.bacc as bacc
import concourse.mybir as mybir
import concourse.tile as tile
from concourse.bass_utils import run_bass_kernel_spmd

A = mybir.AluOpType
F32 = mybir.dt.float32
F16 = mybir.dt.float16
BF16 = mybir.dt.bfloat16
I32 = mybir.dt.int32
U8 = mybir.dt.uint8

SQRT2 = float(np.float32(np.sqrt(2.0)))                # round-to-nearest-log2 threshold

# full problem config
FULL = dict(tok=8192, i=4096, osh=512, r=4, n_cores=8)
B_, S_, O_ = 4, 2048, 4096
TG = 512


def build(tok, i, osh, r, n_cores, reps=1, do_prep=True, do_mm=True, do_bias=True,
          do_drain=True, xdma_once=False, pps_bufs=7, pba_bufs=1,
          xt_bufs=2, os_bufs=4, g0_interleave=True):
    """Build + compile the per-core kernel (SPMD: same program, sharded data).

    reps>1 wraps the whole body in a hardware For_i loop (for benchmarking:
    one dispatch executes the body `reps` times)."""
    nk = i // 128          # contraction k-tiles
    ntg = tok // TG        # 512-token groups

    nc = bacc.Bacc("TRN2", target_bir_lowering=False, debug=False,
                   enable_asserts=False, num_devices=n_cores)
    xt_d = nc.dram_tensor("xtr", [128, ntg, nk, TG], BF16, kind="ExternalInput").ap()
    ori_d = nc.dram_tensor("ori", [128, nk, osh], U8, kind="ExternalInput").ap()
    aux_d = nc.dram_tensor("aux", [128, nk, osh], F32, kind="ExternalInput").ap()
    zp_d = nc.dram_tensor("zp", [1, osh], F32, kind="ExternalInput").ap()
    dl_d = nc.dram_tensor("dl", [1, osh], F32, kind="ExternalInput").ap()
    bt_d = nc.dram_tensor("bt", [r, osh], BF16, kind="ExternalInput").ap()  # loraB shard, pre-transposed
    la_d = nc.dram_tensor("la", [r, i], BF16, kind="ExternalInput").ap()    # loraA
    bias_d = nc.dram_tensor("bias", [1, osh], F32, kind="ExternalInput").ap()
    out_d = nc.dram_tensor("out", [tok, osh], F32, kind="ExternalOutput").ap()

    import contextlib

    with tile.TileContext(nc) as tc:
        with tc.tile_pool(name="const", bufs=1) as cp, \
             tc.tile_pool(name="wres", bufs=1) as wp, \
             tc.tile_pool(name="prep", bufs=2) as pr, \
             tc.tile_pool(name="bpool", bufs=1) as bp, \
             tc.tile_pool(name="pba", bufs=pba_bufs, space="PSUM") as pba, \
             tc.tile_pool(name="pps", bufs=pps_bufs, space="PSUM") as pps, \
             (tc.For_i(0, reps, 1) if reps > 1 else contextlib.nullcontext()):

            # ---- constants + broadcasts
            la_sb = cp.tile([r, i], BF16)
            nc.sync.dma_start(la_sb[:], la_d[:])
            bt_sb = cp.tile([r, osh], BF16)
            nc.sync.dma_start(bt_sb[:], bt_d[:])
            zp_row = cp.tile([1, osh], F32)
            nc.sync.dma_start(zp_row[:], zp_d[:])
            dl_row = cp.tile([1, osh], F32)
            nc.sync.dma_start(dl_row[:], dl_d[:])
            bias_row = cp.tile([1, osh], F32)
            nc.sync.dma_start(bias_row[:], bias_d[:])

            zp_b = cp.tile([128, osh], F32)
            nc.gpsimd.partition_broadcast(zp_b[:], zp_row[:])
            dl_b = cp.tile([128, osh], F32)
            nc.gpsimd.partition_broadcast(dl_b[:], dl_row[:])
            nc.vector.tensor_scalar(dl_b[:], dl_b[:], SQRT2, None, A.mult)
            bias_b = cp.tile([128, 2, osh], F32)
            nc.gpsimd.partition_broadcast(bias_b[:, 0, :], bias_row[:])
            nc.gpsimd.partition_broadcast(bias_b[:, 1, :], bias_row[:])
            # resident transposed weight: [i(128), k, o]
            wT = wp.tile([128, nk, osh], BF16)
            if not do_prep:
                nc.vector.memset(wT[:, 0, 0:1], 0.0)   # touch so reads are legal

            # ---- Phase A: weight prep per i-chunk (7 DVE passes; PE+DMA feed
            # the BA term).  den = (ori + 1e-4) - zp stands in for w_int: the
            # epsilon survives the add (> half-ulp of 255), so den != 0 even
            # when w_int == 0 (keeps recip finite), while q * den still
            # rounds to the exact bf16 weight for w_int != 0.
            # sign*2^round(log2|wu|) is computed as exponent-floor of
            # wu*sqrt2: the sqrt2 mult is fused with the delta add (delta
            # pre-scaled by sqrt2) and the mantissa clear is a fused
            # logical shift-right/left pair.
            # group-0 interleave: issue group 0's matmuls k-outer inside the
            # prep loop so the first output group finishes with Phase A.
            g0_ps = None
            if g0_interleave and do_prep and do_mm:
                xt_g0 = bp.tile([128, nk, TG], BF16, tag="xtg0", bufs=1)
                nc.sync.dma_start(xt_g0[:], xt_d[:, 0, :, :])
                g0_ps0 = pps.tile([128, osh], F32, tag="ps")
                g0_ps1 = pps.tile([128, osh], F32, tag="ps")
                g0_ps2 = pps.tile([128, osh], F32, tag="ps")
                g0_ps3 = pps.tile([128, osh], F32, tag="ps")
                g0_ps = [g0_ps0, g0_ps1, g0_ps2, g0_ps3]

            for k in range(nk if do_prep else 0):
                ba = pba.tile([128, osh], F32, tag="ba")
                nc.tensor.matmul(ba[:], la_sb[:, k * 128:(k + 1) * 128], bt_sb[:],
                                 start=True, stop=True)
                aux_t = pr.tile([128, osh], F32, tag="aux")
                nc.sync.dma_start(aux_t[:], aux_d[:, k, :])
                ori_t = pr.tile([128, osh], U8, tag="ori")
                nc.sync.dma_start(ori_t[:], ori_d[:, k, :])

                den = pr.tile([128, osh], F32, tag="den")
                nc.vector.scalar_tensor_tensor(den[:], ori_t[:], 1e-4,
                                               zp_b[:], A.add, A.subtract)
                rcp = pr.tile([128, osh], F32, tag="rcp")
                nc.vector.reciprocal_approx_fast(rcp[:], den[:])
                num = pr.tile([128, osh], F32, tag="num")
                nc.vector.tensor_tensor(num[:], aux_t[:], ba[:], A.add)
                nc.vector.tensor_tensor(num[:], num[:], rcp[:], A.mult)
                nc.vector.scalar_tensor_tensor(num[:], num[:], SQRT2, dl_b[:],
                                               A.mult, A.add)
                q = pr.tile([128, osh], F32, tag="q")
                nc.vector.tensor_scalar(q[:].bitcast(I32), num[:].bitcast(I32),
                                        23, 23, A.logical_shift_right,
                                        A.logical_shift_left)
                nc.vector.tensor_tensor(wT[:, k, :], q[:], den[:], A.mult)
                if g0_ps is not None:
                    for ts in range(4):
                        nc.tensor.matmul(g0_ps[ts][:], xt_g0[:, k, ts * 128:(ts + 1) * 128],
                                         wT[:, k, :], start=(k == 0), stop=(k == nk - 1))

            # ---- Phase B: stream bf16 x groups, matmul, bias-add drain, store
            xt0 = None
            for tg in range(ntg if do_mm else 0):
                if tg == 0 and g0_ps is not None:
                    for ts in range(4):
                        ps = g0_ps[ts]
                        os_t = bp.tile([128, osh], F32, tag="os", bufs=os_bufs)
                        nc.vector.tensor_tensor(os_t[:], ps[:], bias_b[:, 0, :], A.add)
                        nc.scalar.dma_start(out_d[ts * 128:(ts + 1) * 128, :], os_t[:])
                    continue
                if xdma_once:
                    if xt0 is None:
                        xt0 = bp.tile([128, nk, TG], BF16, tag="xt", bufs=1)
                        nc.sync.dma_start(xt0[:], xt_d[:, 0, :, :])
                    xt = xt0
                else:
                    xt = bp.tile([128, nk, TG], BF16, tag="xt", bufs=xt_bufs)
                    nc.sync.dma_start(xt[:], xt_d[:, tg, :, :])
                for ts in range(TG // 128):
                    tt = tg * (TG // 128) + ts
                    ps = pps.tile([128, osh], F32, tag="ps")
                    for k in range(nk):
                        nc.tensor.matmul(ps[:], xt[:, k, ts * 128:(ts + 1) * 128], wT[:, k, :],
                                         start=(k == 0), stop=(k == nk - 1))
                    if not do_drain:
                        continue
                    os_t = bp.tile([128, osh], F32, tag="os", bufs=os_bufs)
                    if do_bias:
                        nc.vector.tensor_tensor(os_t[:], ps[:], bias_b[:, 0, :], A.add)
                    else:
                        nc.scalar.copy(os_t[:], ps[:])
                    nc.scalar.dma_start(out_d[tt * 128:(tt + 1) * 128, :], os_t[:])

    nc.compile()
    return nc


_CACHE = {}


def _get(cfg_key):
    if cfg_key not in _CACHE:
        _CACHE[cfg_key] = build(**dict(cfg_key))
    return _CACHE[cfg_key]


def make_in_maps(x2d, ori, delta, zp, aux, laA, laB, bias, n_cores, osh):
    import ml_dtypes
    tok, i = x2d.shape
    nk = i // 128
    ntg = tok // TG
    # xtr[p, tg, k, t] = x[tg*TG + t, k*128 + p]
    xtr = np.ascontiguousarray(
        x2d.astype(ml_dtypes.bfloat16).reshape(ntg, TG, nk, 128).transpose(3, 0, 2, 1))
    in_maps = []
    for c in range(n_cores):
        sl = slice(c * osh, (c + 1) * osh)
        # [osh, i] -> [i, osh] -> [p, k, o] with i = k*128 + p
        oriT = ori[sl].T.reshape(nk, 128, osh).transpose(1, 0, 2)
        auxT = aux[sl].T.reshape(nk, 128, osh).transpose(1, 0, 2)
        in_maps.append({
            "xtr": xtr,
            "ori": np.ascontiguousarray(oriT).astype(np.uint8),
            "aux": np.ascontiguousarray(auxT).astype(np.float32),
            "zp": np.ascontiguousarray(zp[sl]).reshape(1, osh),
            "dl": np.ascontiguousarray(delta[sl]).reshape(1, osh),
            "bt": np.ascontiguousarray(laB[sl].T).astype(ml_dtypes.bfloat16),
            "la": laA.astype(ml_dtypes.bfloat16),
            "bias": np.ascontiguousarray(bias[sl]).reshape(1, osh),
        })
    return in_maps


def kernel(x, ori_weight_round, weight_quant_delta, weight_quant_zero_point,
           aux_R, loraA_w, loraB_w, bias, _trace=False):
    cfg = FULL
    n_cores, osh = cfg["n_cores"], cfg["osh"]
    x2d = np.ascontiguousarray(np.asarray(x, dtype=np.float32).reshape(cfg["tok"], cfg["i"]))
    nc = _get(tuple(sorted(cfg.items())))
    in_maps = make_in_maps(
        x2d,
        np.asarray(ori_weight_round, np.float32),
        np.asarray(weight_quant_delta, np.float32),
        np.asarray(weight_quant_zero_point, np.float32),
        np.asarray(aux_R, np.float32),
        np.asarray(loraA_w, np.float32),
        np.asarray(loraB_w, np.float32),
        np.asarray(bias, np.float32),
        n_cores, osh)
    res = run_bass_kernel_spmd(nc, in_maps, core_ids=list(range(n_cores)), trace=_trace)
    out = np.concatenate([res.results[c]["out"] for c in range(n_cores)], axis=1)
    out = out.reshape(B_, S_, O_)
    if _trace:
        return out, res
    return out


# revision 44
# speedup vs baseline: 3.7522x; 3.7522x over previous
"""IntLoRA-SHIFT fused kernel for Trainium2 (8 NeuronCores, tensor-parallel on out_features).

Math (per reference):
    w_int  = ori_weight_round - zero_point                    [O, I]
    lora   = (aux_R + loraB @ loraA) / where(w_int==0, 1, w_int)
    wu     = delta + lora
    weight = sign(wu) * 2^round(log2|wu|) * w_int
    out    = x @ weight.T + bias

Kernel strategy per core (O sharded 8 x 512):
  Everything lives in [i(partition), o(free)] layout so weight prep emits the
  k-tile-transposed weight wT directly (no on-device transposes).  The host
  pre-transposes ori (uint8) and aux (f32) per core; per-o params
  (zero_point/delta/bias) are partition-broadcast once into [128, osh] tiles.

  Phase A (per i-chunk k, [128, 512] tiles, 7 DVE passes, all other engines
  free):
    PE:  ba = (loraB@loraA)^T slice  (K=4 bf16 matmul into PSUM)
    DVE: den = (ori + 1e-4) - zp_b      (epsilon keeps den != 0 at w_int==0;
         q*den still rounds to the exact bf16 weight for w_int != 0)
    DVE: rcp ~= 1/den (reciprocal_approx_fast, ~18 bits -- only the rounded
         log2 of the quotient is needed)
    DVE: num = (aux + ba) * rcp ; num = num*sqrt2 + delta*sqrt2   (fused stt;
         exponent-floor of wu*sqrt2 == round(log2|wu|))
    DVE: q = bits(num) >>23 <<23 (fused shift pair: sign * 2^round(log2|wu|))
    DVE: wT[:,k,:] = q * den  (exact in bf16: |w_int|<=255 has 8 significand bits)

  Phase B: host-transposed bf16 x streamed as [128, nk, 512] groups (contig
  32KB/partition DMAs, triple-buffered), 32 bf16 matmuls per 128-token tile
  accumulate in PSUM rotating over 7 banks (bank-level ILP on the PE is worth
  ~2x vs 4 banks), drained by DVE bias-adds and stored with HWDGE DMAs.
  All matmuls are bf16: no fp32/bf16 PE mode switches anywhere.
"""
import os
import sys

for _p in ("/root/.axon_site", "/root/.axon_site/_ro/trn_rl_repo", "/root/.axon_site/_ro/pypackages", "/opt/trn_rl_repo"):
    if os.path.isdir(_p) and _p not in sys.path:
        sys.path.append(_p)

import numpy as np

import concourse.bacc as bacc
import concourse.mybir as mybir
import concourse.tile as tile
from concourse.bass_utils import run_bass_kernel_spmd

A = mybir.AluOpType
F32 = mybir.dt.float32
F16 = mybir.dt.float16
BF16 = mybir.dt.bfloat16
I32 = mybir.dt.int32
U8 = mybir.dt.uint8

SQRT2 = float(np.float32(np.sqrt(2.0)))                # round-to-nearest-log2 threshold

# full problem config
FULL = dict(tok=8192, i=4096, osh=512, r=4, n_cores=8)
B_, S_, O_ = 4, 2048, 4096
TG = 512


def build(tok, i, osh, r, n_cores, reps=1, do_prep=True, do_mm=True, do_bias=True,
          do_drain=True, xdma_once=False, pps_bufs=7, pba_bufs=1,
          xt_bufs=2, os_bufs=4, g0_interleave=True, prep_bufs=2):
    """Build + compile the per-core kernel (SPMD: same program, sharded data).

    reps>1 wraps the whole body in a hardware For_i loop (for benchmarking:
    one dispatch executes the body `reps` times)."""
    nk = i // 128          # contraction k-tiles
    ntg = tok // TG        # 512-token groups

    nc = bacc.Bacc("TRN2", target_bir_lowering=False, debug=False,
                   enable_asserts=False, num_devices=n_cores)
    xt_d = nc.dram_tensor("xtr", [128, ntg, nk, TG], BF16, kind="ExternalInput").ap()
    ori_d = nc.dram_tensor("ori", [128, nk, osh], U8, kind="ExternalInput").ap()
    aux_d = nc.dram_tensor("aux", [128, nk, osh], F32, kind="ExternalInput").ap()
    zp_d = nc.dram_tensor("zp", [1, osh], F32, kind="ExternalInput").ap()
    dl_d = nc.dram_tensor("dl", [1, osh], F32, kind="ExternalInput").ap()
    bt_d = nc.dram_tensor("bt", [r, osh], BF16, kind="ExternalInput").ap()  # loraB shard, pre-transposed
    la_d = nc.dram_tensor("la", [r, i], BF16, kind="ExternalInput").ap()    # loraA
    bias_d = nc.dram_tensor("bias", [1, osh], F32, kind="ExternalInput").ap()
    out_d = nc.dram_tensor("out", [tok, osh], F32, kind="ExternalOutput").ap()

    import contextlib

    with tile.TileContext(nc) as tc:
        with tc.tile_pool(name="const", bufs=1) as cp, \
             tc.tile_pool(name="wres", bufs=1) as wp, \
             tc.tile_pool(name="prep", bufs=prep_bufs) as pr, \
             tc.tile_pool(name="bpool", bufs=1) as bp, \
             tc.tile_pool(name="pba", bufs=pba_bufs, space="PSUM") as pba, \
             tc.tile_pool(name="pps", bufs=pps_bufs, space="PSUM") as pps, \
             (tc.For_i(0, reps, 1) if reps > 1 else contextlib.nullcontext()):

            # ---- constants + broadcasts
            la_sb = cp.tile([r, i], BF16)
            nc.sync.dma_start(la_sb[:], la_d[:])
            bt_sb = cp.tile([r, osh], BF16)
            nc.sync.dma_start(bt_sb[:], bt_d[:])
            zp_row = cp.tile([1, osh], F32)
            nc.sync.dma_start(zp_row[:], zp_d[:])
            dl_row = cp.tile([1, osh], F32)
            nc.sync.dma_start(dl_row[:], dl_d[:])
            bias_row = cp.tile([1, osh], F32)
            nc.sync.dma_start(bias_row[:], bias_d[:])

            zp_b = cp.tile([128, osh], F32)
            nc.gpsimd.partition_broadcast(zp_b[:], zp_row[:])
            dl_b = cp.tile([128, osh], F32)
            nc.gpsimd.partition_broadcast(dl_b[:], dl_row[:])
            nc.vector.tensor_scalar(dl_b[:], dl_b[:], SQRT2, None, A.mult)
            bias_b = cp.tile([128, 2, osh], F32)
            nc.gpsimd.partition_broadcast(bias_b[:, 0, :], bias_row[:])
            nc.gpsimd.partition_broadcast(bias_b[:, 1, :], bias_row[:])
            # resident transposed weight: [i(128), k, o]
            wT = wp.tile([128, nk, osh], BF16)
            if not do_prep:
                nc.vector.memset(wT[:, 0, 0:1], 0.0)   # touch so reads are legal

            # ---- Phase A: weight prep per i-chunk (7 DVE passes; PE+DMA feed
            # the BA term).  den = (ori + 1e-4) - zp stands in for w_int: the
            # epsilon survives the add (> half-ulp of 255), so den != 0 even
            # when w_int == 0 (keeps recip finite), while q * den still
            # rounds to the exact bf16 weight for w_int != 0.
            # sign*2^round(log2|wu|) is computed as exponent-floor of
            # wu*sqrt2: the sqrt2 mult is fused with the delta add (delta
            # pre-scaled by sqrt2) and the mantissa clear is a fused
            # logical shift-right/left pair.
            # group-0 interleave: issue group 0's matmuls k-outer inside the
            # prep loop so the first output group finishes with Phase A.
            g0_ps = None
            if g0_interleave and do_prep and do_mm:
                xt_g0 = bp.tile([128, nk, TG], BF16, tag="xtg0", bufs=1)
                nc.sync.dma_start(xt_g0[:], xt_d[:, 0, :, :])
                g0_ps0 = pps.tile([128, osh], F32, tag="ps")
                g0_ps1 = pps.tile([128, osh], F32, tag="ps")
                g0_ps2 = pps.tile([128, osh], F32, tag="ps")
                g0_ps3 = pps.tile([128, osh], F32, tag="ps")
                g0_ps = [g0_ps0, g0_ps1, g0_ps2, g0_ps3]

            for k in range(nk if do_prep else 0):
                ba = pba.tile([128, osh], F32, tag="ba")
                nc.tensor.matmul(ba[:], la_sb[:, k * 128:(k + 1) * 128], bt_sb[:],
                                 start=True, stop=True)
                aux_t = pr.tile([128, osh], F32, tag="aux")
                nc.sync.dma_start(aux_t[:], aux_d[:, k, :])
                ori_t = pr.tile([128, osh], U8, tag="ori")
                nc.sync.dma_start(ori_t[:], ori_d[:, k, :])

                den = pr.tile([128, osh], F32, tag="den")
                nc.vector.scalar_tensor_tensor(den[:], ori_t[:], 1e-4,
                                               zp_b[:], A.add, A.subtract)
                rcp = pr.tile([128, osh], F32, tag="rcp")
                nc.vector.reciprocal_approx_fast(rcp[:], den[:])
                num = pr.tile([128, osh], F32, tag="num")
                nc.vector.tensor_tensor(num[:], aux_t[:], ba[:], A.add)
                nc.vector.tensor_tensor(num[:], num[:], rcp[:], A.mult)
                nc.vector.scalar_tensor_tensor(num[:], num[:], SQRT2, dl_b[:],
                                               A.mult, A.add)
                q = pr.tile([128, osh], F32, tag="q")
                nc.vector.tensor_scalar(q[:].bitcast(I32), num[:].bitcast(I32),
                                        23, 23, A.logical_shift_right,
                                        A.logical_shift_left)
                nc.vector.tensor_tensor(wT[:, k, :], q[:], den[:], A.mult)
                if g0_ps is not None:
                    for ts in range(4):
                        nc.tensor.matmul(g0_ps[ts][:], xt_g0[:, k, ts * 128:(ts + 1) * 128],
                                         wT[:, k, :], start=(k == 0), stop=(k == nk - 1))

            # ---- Phase B: stream bf16 x groups, matmul, bias-add drain, store
            xt0 = None
            for tg in range(ntg if do_mm else 0):
                if tg == 0 and g0_ps is not None:
                    for ts in range(4):
                        ps = g0_ps[ts]
                        os_t = bp.tile([128, osh], F32, tag="os", bufs=os_bufs)
                        nc.vector.tensor_tensor(os_t[:], ps[:], bias_b[:, 0, :], A.add)
                        nc.scalar.dma_start(out_d[ts * 128:(ts + 1) * 128, :], os_t[:])
                    continue
                if xdma_once:
                    if xt0 is None:
                        xt0 = bp.tile([128, nk, TG], BF16, tag="xt", bufs=1)
                        nc.sync.dma_start(xt0[:], xt_d[:, 0, :, :])
                    xt = xt0
                else:
                    xt = bp.tile([128, nk, TG], BF16, tag="xt", bufs=xt_bufs)
                    nc.sync.dma_start(xt[:], xt_d[:, tg, :, :])
                for ts in range(TG // 128):
                    tt = tg * (TG // 128) + ts
                    ps = pps.tile([128, osh], F32, tag="ps")
                    for k in range(nk):
                        nc.tensor.matmul(ps[:], xt[:, k, ts * 128:(ts + 1) * 128], wT[:, k, :],
                                         start=(k == 0), stop=(k == nk - 1))
                    if not do_drain:
                        continue
                    os_t = bp.tile([128, osh], F32, tag="os", bufs=os_bufs)
                    if do_bias:
                        nc.vector.tensor_tensor(os_t[:], ps[:], bias_b[:, 0, :], A.add)
                    else:
                        nc.scalar.copy(os_t[:], ps[:])
                    nc.scalar.dma_start(out_d[tt * 128:(tt + 1) * 128, :], os_t[:])

    nc.compile()
    return nc


_CACHE = {}


def _get(cfg_key):
    if cfg_key not in _CACHE:
        _CACHE[cfg_key] = build(**dict(cfg_key))
    return _CACHE[cfg_key]


def make_in_maps(x2d, ori, delta, zp, aux, laA, laB, bias, n_cores, osh):
    import ml_dtypes
    tok, i = x2d.shape
    nk = i // 128
    ntg = tok // TG
    # xtr[p, tg, k, t] = x[tg*TG + t, k*128 + p]
    xtr = np.ascontiguousarray(
        x2d.astype(ml_dtypes.bfloat16).reshape(ntg, TG, nk, 128).transpose(3, 0, 2, 1))
    in_maps = []
    for c in range(n_cores):
        sl = slice(c * osh, (c + 1) * osh)
        # [osh, i] -> [i, osh] -> [p, k, o] with i = k*128 + p
        oriT = ori[sl].T.reshape(nk, 128, osh).transpose(1, 0, 2)
        auxT = aux[sl].T.reshape(nk, 128, osh).transpose(1, 0, 2)
        in_maps.append({
            "xtr": xtr,
            "ori": np.ascontiguousarray(oriT).astype(np.uint8),
            "aux": np.ascontiguousarray(auxT).astype(np.float32),
            "zp": np.ascontiguousarray(zp[sl]).reshape(1, osh),
            "dl": np.ascontiguousarray(delta[sl]).reshape(1, osh),
            "bt": np.ascontiguousarray(laB[sl].T).astype(ml_dtypes.bfloat16),
            "la": laA.astype(ml_dtypes.bfloat16),
            "bias": np.ascontiguousarray(bias[sl]).reshape(1, osh),
        })
    return in_maps


def kernel(x, ori_weight_round, weight_quant_delta, weight_quant_zero_point,
           aux_R, loraA_w, loraB_w, bias, _trace=False):
    cfg = FULL
    n_cores, osh = cfg["n_cores"], cfg["osh"]
    x2d = np.ascontiguousarray(np.asarray(x, dtype=np.float32).reshape(cfg["tok"], cfg["i"]))
    nc = _get(tuple(sorted(cfg.items())))
    in_maps = make_in_maps(
        x2d,
        np.asarray(ori_weight_round, np.float32),
        np.asarray(weight_quant_delta, np.float32),
        np.asarray(weight_quant_zero_point, np.float32),
        np.asarray(aux_R, np.float32),
        np.asarray(loraA_w, np.float32),
        np.asarray(loraB_w, np.float32),
        np.asarray(bias, np.float32),
        n_cores, osh)
    res = run_bass_kernel_spmd(nc, in_maps, core_ids=list(range(n_cores)), trace=_trace)
    out = np.concatenate([res.results[c]["out"] for c in range(n_cores)], axis=1)
    out = out.reshape(B_, S_, O_)
    if _trace:
        return out, res
    return out
